# revision 1
# baseline (speedup 1.0000x reference)
"""Trainium2 Bass kernel for CausalGNNLayer (per-node-type Linear, MoE-style routing).

Semantics (matching the reference):
    out[n, :] = x[n, :] @ W[node_types[n]].T + b[node_types[n]]
edge_index is unused by the op.

Strategy:
- Host-side routing-aware sharding: stable-sort nodes by type, split each
  type's node list into two halves -> 8 groups (4 types x 2 cores).
- x in fp8 (e3m4), W in bf16 — a mixed-dtype matmul the PE runs at full
  rate — with fp32 PSUM accumulation and bf16 output storage.  This cuts
  HBM traffic ~3.3x vs fp32 at rel-err ~1.2e-2 (gate 2e-2).
- Weight-stationary matmul schedule (out.T layout): psum[o_blk, nodes] +=
  w[k, o_blk].T @ xT[k, nodes].  The stationary operand (a 128x128 tile of
  W.T) is reused across all chunks of a group, so LDWEIGHTS amortizes and
  hides under the moving stream; matmuls run back-to-back at the tensor
  engine's streaming rate (~N/2.4GHz).
- Variable chunk grid: a 128-node mini chunk first (compute starts after
  ~450KB of DMA; its LDWEIGHTS-paced matmuls are immune to the clock ramp),
  512-node chunks in pair groups, and a small final chunk sized so P just
  covers the largest shard (minimizes padded FLOPs).
- PE clock warmup: the tensor engine ramps 0.65->2.4 GHz only under real
  MATMUL execution (weight loads do not ramp it), so dummy matmuls on
  garbage SBUF data — no DMA dependencies — absorb the ~3us ramp while the
  first data DMAs land; every real 512-wide matmul then runs at the 216ns
  streaming floor.
- Drain (bias add + fp32->bf16 downcast) alternates between the Vector and
  the otherwise-idle Scalar engine; in out.T layout the bias is a
  per-partition scalar, which both engines support natively.
- Host scatters the 8 bf16 output shards back into the full [N, 512] fp32
  output.
"""

import numpy as np
import ml_dtypes
from contextlib import ExitStack

import concourse.bass as bass
import concourse.mybir as mybir
import concourse.tile as tile
from concourse.bass_utils import run_bass_kernel_spmd

N_CORES = 8
IN_CH = 512
OUT_CH = 512
NUM_TYPES = 4
P_BLK = 128          # partition count
KT = IN_CH // P_BLK  # 4 contraction tiles
CHUNK_N = 512        # main chunk width == psum bank capacity (fp32)
MINI_N = 128         # first chunk width (early compute start)
SGRP = 2             # chunks per stationary-reuse group
XBUFS = 10           # x-chunk prefetch depth
PSBUFS = 4           # psum ring: 4 tiles x 2 banks = all 8 banks
OBUFS = 4            # output staging depth
WARMUP_LDW = 10      # dummy LDWEIGHTS to ramp the PE p-state during DMA wait

# Set by test harness to capture HW profile; kernel works without it.
TRACE = False
LAST_RESULTS = None

_compile_cache: dict = {}

_legal_nop_counter = [0]


def _legalize_waits(nc: bass.Bass) -> None:
    """This walrus codegen only encodes ONE sync wait per engine instruction.
    Tile's scheduler attaches several.  Split: hoist all-but-one wait of any
    multi-wait instruction into preceding same-engine NoOps (one wait each) —
    semantically identical (the engine stalls on each wait in program order)."""
    for fn in nc.m.functions:
        for blk in fn.blocks:
            insts = blk.instructions
            out = []
            changed = False
            for inst in insts:
                si = inst.sync_info
                waits = list(si.on_wait) if si is not None and si.on_wait else []
                if len(waits) > 1:
                    changed = True
                    for w in waits[:-1]:
                        _legal_nop_counter[0] += 1
                        nop = mybir.InstNoOp(
                            name=f"waitsplit-{_legal_nop_counter[0]}",
                            ins=[],
                            outs=[],
                            engine=inst.engine,
                        )
                        nop.sync_info = mybir.SyncInfo(on_wait=[w], on_update=[])
                        out.append(nop)
                    inst.sync_info = mybir.SyncInfo(
                        on_wait=[waits[-1]], on_update=list(si.on_update or [])
                    )
                out.append(inst)
            if changed:
                blk.instructions = out


def _plan(P_needed: int):
    """Chunk widths [MINI_N, 512, ..., 512, final] covering >= P_needed,
    and stationary-reuse groups (graded sizes over the 512-chunks)."""
    rem = P_needed - MINI_N
    n512 = max(0, (rem - 1) // CHUNK_N)
    final = rem - n512 * CHUNK_N
    final = ((final + 63) // 64) * 64  # pad to 64 for sane DMA runs
    widths = [MINI_N] + [CHUNK_N] * n512 + ([final] if final else [])
    # groups: mini alone, then pairs (SGRP=2) over the 512s — pairs keep the
    # 4-deep psum ring's turnaround budget (3 oblk bursts ~5us) well above
    # the drain latency, and the x-DMA stream ahead of chunk needs; final
    # chunk alone.
    groups = [[0]]
    graded = [1]
    idx = 1
    gi = 0
    while idx < 1 + n512:
        s = graded[gi] if gi < len(graded) else SGRP
        s = min(s, 1 + n512 - idx)
        groups.append(list(range(idx, idx + s)))
        idx += s
        gi += 1
    if final:
        groups.append([1 + n512])
    offs = np.concatenate([[0], np.cumsum(widths)]).astype(int)
    return widths, list(offs[:-1]), int(offs[-1]), groups


def _build_bass(plan_key) -> bass.Bass:
    widths, offs, P, groups = plan_key
    nc = bass.Bass("TRN2")
    f32 = mybir.dt.float32
    bf16 = mybir.dt.bfloat16
    f8 = mybir.dt.float8e3

    OBLKS = OUT_CH // P_BLK  # 4
    nchunks = len(widths)

    # x stored chunk-major, each chunk a [128, KT, 512] block (tail columns
    # of narrow chunks unused); slicing [:, :, :w] keeps 512-wide chunk DMAs
    # as single 4KB contiguous runs per partition.
    xT = nc.dram_tensor(
        "xT", [nchunks, P_BLK, KT, CHUNK_N], f8, kind="ExternalInput"
    )
    w = nc.dram_tensor("w", [IN_CH, OUT_CH], bf16, kind="ExternalInput")
    # bias2[p, oblk] = b[oblk*128 + p]
    bias2 = nc.dram_tensor("bias2", [P_BLK, OBLKS], f32, kind="ExternalInput")
    # outT[oblk, p, n] = out[n, oblk*128+p]
    out = nc.dram_tensor("out", [OBLKS, P_BLK, P], bf16, kind="ExternalOutput")

    w_v = w.ap().rearrange("(k p) o -> p k o", p=P_BLK)

    with ExitStack() as ctx:
        tc = ctx.enter_context(tile.TileContext(nc))
        wp = ctx.enter_context(tc.tile_pool(name="w", bufs=1))
        warmp = ctx.enter_context(tc.tile_pool(name="warm", bufs=1))
        actp = ctx.enter_context(tc.tile_pool(name="actw", bufs=1))
        bp = ctx.enter_context(tc.tile_pool(name="b", bufs=1))
        xp = ctx.enter_context(tc.tile_pool(name="x", bufs=XBUFS))
        pp = ctx.enter_context(tc.tile_pool(name="ps", bufs=PSBUFS, space="PSUM"))
        op = ctx.enter_context(tc.tile_pool(name="o", bufs=OBUFS))

        # PE p-state warmup: the tensor engine ramps 0.65->1.2->2.4 GHz with
        # ~3us of continuous MATMUL execution (weight loads alone do not
        # ramp the clock).  Dummy matmuls on garbage operands — no DMA
        # dependencies — finish the ramp while the first data DMAs land, so
        # real matmuls start at full clock.
        warm_sb = warmp.tile([P_BLK, 4 * P_BLK], bf16)
        nc.gpsimd.memset(warm_sb[:], 0)
        for _ in range(WARMUP_LDW):
            nc.tensor.ldweights(warm_sb[:, 0:P_BLK])
        ps_warm = pp.tile([P_BLK, CHUNK_N], f32, tag="ps")
        for _ in range(7):
            nc.tensor.matmul(
                ps_warm[:],
                lhsT=warm_sb[:, 0:P_BLK],
                rhs=warm_sb[:],
                start=True,
                stop=True,
            )
        # ACT-table preload (~1.3us one-time) so the first real drain on the
        # scalar engine isn't delayed; separate tile so it can't serialize
        # ahead of the LDWEIGHTS warmup.
        act_sb = actp.tile([1, 1], f32)
        nc.gpsimd.memset(act_sb[:], 0)
        nc.scalar.add(act_sb[:], act_sb[:], 0.0)

        x_tiles: dict[int, object] = {}

        def fetch_chunk(c: int):
            if c not in x_tiles:
                wd = widths[c]
                t = xp.tile([P_BLK, KT, wd], f8, tag="x")
                nc.sync.dma_start(t[:], xT.ap()[c][:, :, 0:wd])
                x_tiles[c] = t

        w_sb = wp.tile([P_BLK, KT, OUT_CH], bf16)
        # issue order: w k0/k1, mini chunk, w k2/k3 — the first matmuls need
        # only ~450KB of DMA; the mini group's k-outer order gives the second
        # w half another ~1.3us to land.
        nc.sync.dma_start(w_sb[:, 0:2, :], w_v[:, 0:2, :])
        fetch_chunk(0)
        nc.sync.dma_start(w_sb[:, 2:4, :], w_v[:, 2:4, :])
        b_sb = bp.tile([P_BLK, OBLKS], f32)
        nc.sync.dma_start(b_sb[:], bias2.ap())

        drain_flip = [0]
        for gi, grp in enumerate(groups):
            for c in grp:
                fetch_chunk(c)
            if gi + 1 < len(groups):
                for c in groups[gi + 1]:
                    fetch_chunk(c)
            gw = sum(widths[c] for c in grp)
            goff = offs[grp[0]]
            if gi == 0:
                # Mini group: k-outer over all 4 oblk psum slices so each w
                # k-slice is first needed ~0.7us after the previous one.
                # Slices sit at 512-element offsets (psum-bank aligned).
                psA = pp.tile([P_BLK, 2 * CHUNK_N], f32, tag="ps")
                psB = pp.tile([P_BLK, 2 * CHUNK_N], f32, tag="ps")
                mslice = lambda oblk: (psA if oblk < 2 else psB)[
                    :, (oblk % 2) * CHUNK_N : (oblk % 2) * CHUNK_N + gw
                ]
                for k in range(KT):
                    for oblk in range(OBLKS):
                        nc.tensor.matmul(
                            mslice(oblk),
                            lhsT=w_sb[:, k, oblk * P_BLK : (oblk + 1) * P_BLK],
                            rhs=x_tiles[grp[0]][:, k, :],
                            start=(k == 0),
                            stop=(k == KT - 1),
                        )
                for oblk in range(OBLKS):
                    o_sb = op.tile([P_BLK, gw], bf16, tag="o")
                    bias_ap = b_sb[:, oblk : oblk + 1]
                    if drain_flip[0] % 2 == 0:
                        nc.vector.tensor_scalar_add(o_sb[:], mslice(oblk), bias_ap)
                    else:
                        nc.scalar.add(o_sb[:], mslice(oblk), bias_ap)
                    drain_flip[0] += 1
                    nc.sync.dma_start(out.ap()[oblk, :, goff : goff + gw], o_sb[:])
                continue
            for oblk in range(OBLKS):
                ps = pp.tile([P_BLK, gw], f32, tag="ps")
                for k in range(KT):
                    lhsT = w_sb[:, k, oblk * P_BLK : (oblk + 1) * P_BLK]
                    loc = 0
                    for c in grp:
                        nc.tensor.matmul(
                            ps[:, loc : loc + widths[c]],
                            lhsT=lhsT,
                            rhs=x_tiles[c][:, k, :],
                            start=(k == 0),
                            stop=(k == KT - 1),
                        )
                        loc += widths[c]
                o_sb = op.tile([P_BLK, gw], bf16, tag="o")
                bias_ap = b_sb[:, oblk : oblk + 1]
                if drain_flip[0] % 2 == 0:
                    nc.vector.tensor_scalar_add(o_sb[:], ps[:], bias_ap)
                else:
                    nc.scalar.add(o_sb[:], ps[:], bias_ap)
                drain_flip[0] += 1
                nc.sync.dma_start(out.ap()[oblk, :, goff : goff + gw], o_sb[:])
    _legalize_waits(nc)
    return nc


def _get_compiled(plan_key) -> bass.Bass:
    key = (tuple(plan_key[0]), plan_key[2])
    if key not in _compile_cache:
        _compile_cache[key] = _build_bass(plan_key)
    return _compile_cache[key]


def kernel(x, edge_index, node_types, W, b):
    global LAST_RESULTS
    x = np.asarray(x, dtype=np.float32)
    nt = np.asarray(node_types).astype(np.int64)
    W = np.asarray(W, dtype=np.float32)
    b = np.asarray(b, dtype=np.float32)
    N = x.shape[0]

    # Route nodes: stable sort by type, split each type across 2 cores.
    order = np.argsort(nt, kind="stable")
    counts = np.bincount(nt, minlength=NUM_TYPES)
    shards = []
    start = 0
    for t in range(NUM_TYPES):
        c = int(counts[t])
        idx = order[start : start + c]
        start += c
        h = (c + 1) // 2
        shards.append(idx[:h])
        shards.append(idx[h:])

    P_needed = max(1, max(len(g) for g in shards))
    plan = _plan(P_needed)
    widths, offs, P, groups = plan
    nchunks = len(widths)

    nc = _get_compiled(plan)

    in_maps = []
    for gi, g in enumerate(shards):
        t = gi // 2
        xs = np.zeros((P, IN_CH), np.float32)
        if len(g):
            xs[: len(g)] = x[g]
        xsT = xs.T.astype(ml_dtypes.float8_e3m4)  # [512, P]
        xbuf = np.zeros((nchunks, P_BLK, KT, CHUNK_N), ml_dtypes.float8_e3m4)
        for c in range(nchunks):
            wd = widths[c]
            seg = xsT[:, offs[c] : offs[c] + wd].reshape(KT, P_BLK, wd)
            xbuf[c, :, :, :wd] = seg.transpose(1, 0, 2)
        in_maps.append(
            {
                "xT": xbuf,
                "w": np.ascontiguousarray(W[t].T).astype(ml_dtypes.bfloat16),
                "bias2": np.ascontiguousarray(
                    b[t].reshape(4, P_BLK).T.astype(np.float32)
                ),
            }
        )

    res = run_bass_kernel_spmd(nc, in_maps, list(range(N_CORES)), trace=TRACE)
    LAST_RESULTS = res

    out = np.empty((N, OUT_CH), np.float32)
    for gi, g in enumerate(shards):
        if len(g):
            # outT [4, 128, P] -> [P, 512] node-major
            o = res.results[gi]["out"].reshape(OUT_CH, P).T.astype(np.float32)
            out[g] = o[: len(g)]
    return out



# revision 2
# speedup vs baseline: 1.1775x; 1.1775x over previous
"""Trainium2 Bass kernel for CausalGNNLayer (per-node-type Linear, MoE-style routing).

Semantics (matching the reference):
    out[n, :] = x[n, :] @ W[node_types[n]].T + b[node_types[n]]
edge_index is unused by the op.

Strategy (v2 — fp8e4 DoubleRow, P0-downclock-aware):
- Host-side routing-aware sharding: stable-sort nodes by type, split each
  type's node list into two halves -> 8 groups (4 types x 2 cores).
- Measured on this part: a sustained 8-core bf16-PE stream trips the chip's
  P0 power state and pins the PE at 2.0 GHz (259 ns / 512-wide matmul);
  an fp8e4 DoubleRow stream (2 MACs/PE/cycle, K=256 per instruction) stays
  at 2.4 GHz (216 ns).  DoubleRow needs both operands in fp8-e4m3, whose
  3-bit mantissa alone would blow the 2e-2 gate (measured 3.8e-2), so we
  run TWO DoubleRow streams accumulated in one PSUM group:
      y*2^17 = m1 @ Wq + u @ V,  where
      m1 = Q((1-a)*x*32),  xhat = m1/((1-a)*32),  xlo = x - xhat
      u  = Q((xlo + a*xhat)*32)
      Wq = Q(W*4096),      V = Q((What + (W-What)/a)*4096),  a = 1/8
  The correction stream cancels both operands' quantization error to first
  order: measured end-to-end rel err 7.2e-3 (vs 1.18e-2 for the old
  bf16xfp8e3 kernel).  Same instruction count as the bf16 schedule, but at
  216 ns/instr instead of 259: steady-state ~85 us vs ~102 us.
  Scales are powers of two (32*4096 = 2^17); bias is pre-scaled by 2^17 on
  host and the output divided by 2^17 after the run, so the device drain is
  still a plain add+downcast.
- Group-major tiles: 1024 nodes per x tile / psum burst / output tile, ONE
  dma_start in and ONE out per group (each dma_start costs ~700 ns of sync-
  queue DIRECT2D issue; the old 4-outs-per-group schedule kept the sync
  queue 94% busy and serialized the tail).
- out2 dram layout [128, 4, P] (partition-major) so the 4 psum drains of a
  group land in one SBUF tile and leave as one strided DMA.
- Warmup: dummy DoubleRow matmuls on a zeroed tile ramp the HAM clock gate
  (cold 1.2 GHz for ~3.4 us) while the first data DMAs land; mini 128-node
  first chunk starts real work on ~72 KB of data.
- Drain (bias add + fp32->bf16 downcast) alternates Vector/Scalar engines.
- Host scatters the 8 output shards back into the full [N, 512] fp32 output.
"""

import numpy as np
import ml_dtypes
from contextlib import ExitStack

import concourse.bass as bass
import concourse.mybir as mybir
import concourse.tile as tile
from concourse.bass_utils import run_bass_kernel_spmd

N_CORES = 8
IN_CH = 512
OUT_CH = 512
NUM_TYPES = 4
P_BLK = 128
OBLKS = OUT_CH // P_BLK   # 4
MINI_N = 128              # first chunk width (early compute start)
GRP_N = 1024              # steady group width (one x tile / one out tile)
XBUFS = 5                 # x group-tile prefetch depth (8KB/partition each)
PSBUFS = 4                # psum ring: 4 tiles x 2 banks = all 8 banks
OBUFS = 3                 # output staging depth (8KB/partition each)
WARMUP_MM = 12            # dummy DoubleRow matmuls to ramp the clock gate

ALPHA = 0.125
SM = 32.0                 # moving-operand scale
SW = 4096.0               # stationary-operand scale
SCALE = SM * SW           # 2^17

TRACE = False
LAST_RESULTS = None

_compile_cache: dict = {}

_legal_nop_counter = [0]


def _legalize_waits(nc: bass.Bass) -> None:
    """This walrus codegen only encodes ONE sync wait per engine instruction.
    Tile's scheduler attaches several.  Split: hoist all-but-one wait of any
    multi-wait instruction into preceding same-engine NoOps (one wait each) —
    semantically identical (the engine stalls on each wait in program order)."""
    for fn in nc.m.functions:
        for blk in fn.blocks:
            insts = blk.instructions
            out = []
            changed = False
            for inst in insts:
                si = inst.sync_info
                waits = list(si.on_wait) if si is not None and si.on_wait else []
                if len(waits) > 1:
                    changed = True
                    for w in waits[:-1]:
                        _legal_nop_counter[0] += 1
                        nop = mybir.InstNoOp(
                            name=f"waitsplit-{_legal_nop_counter[0]}",
                            ins=[],
                            outs=[],
                            engine=inst.engine,
                        )
                        nop.sync_info = mybir.SyncInfo(on_wait=[w], on_update=[])
                        out.append(nop)
                    inst.sync_info = mybir.SyncInfo(
                        on_wait=[waits[-1]], on_update=list(si.on_update or [])
                    )
                out.append(inst)
            if changed:
                blk.instructions = out


def _plan(P_needed: int):
    """Chunk widths [MINI_N, 1024, ..., 1024, final] covering >= P_needed."""
    rem = P_needed - MINI_N
    nfull = max(0, (rem - 1) // GRP_N)
    final = rem - nfull * GRP_N
    final = ((final + 63) // 64) * 64
    widths = [MINI_N] + [GRP_N] * nfull + ([final] if final else [])
    offs = np.concatenate([[0], np.cumsum(widths)]).astype(int)
    return widths, list(offs[:-1]), int(offs[-1])


def _build_bass(plan_key) -> bass.Bass:
    widths, offs, P = plan_key
    nc = bass.Bass("TRN2")
    f32 = mybir.dt.float32
    bf16 = mybir.dt.bfloat16
    f8e4 = mybir.dt.float8e4
    DR = mybir.MatmulPerfMode.DoubleRow

    nchunks = len(widths)

    # xT[c, p, s, j, i, n]: stream s, kk-pair j, plane i, node n of chunk c;
    # contraction index kappa = j*256 + i*128 + p.
    xT = nc.dram_tensor(
        "xT", [nchunks, P_BLK, 2, 2, 2, GRP_N], f8e4, kind="ExternalInput"
    )
    # w8[p, s, j, i, o*128+m]
    w8 = nc.dram_tensor("w8", [P_BLK, 2, 2, 2, OUT_CH], f8e4, kind="ExternalInput")
    # bias2[p, oblk] = b[oblk*128 + p] * 2^17
    bias2 = nc.dram_tensor("bias2", [P_BLK, OBLKS], f32, kind="ExternalInput")
    # out2[p, oblk, n] = (y[n, oblk*128+p] * 2^17) as bf16
    out2 = nc.dram_tensor("out2", [P_BLK, OBLKS, P], bf16, kind="ExternalOutput")

    with ExitStack() as ctx:
        tc = ctx.enter_context(tile.TileContext(nc))
        wp = ctx.enter_context(tc.tile_pool(name="w", bufs=1))
        warmp = ctx.enter_context(tc.tile_pool(name="warm", bufs=1))
        actp = ctx.enter_context(tc.tile_pool(name="actw", bufs=1))
        bp = ctx.enter_context(tc.tile_pool(name="b", bufs=1))
        xp = ctx.enter_context(tc.tile_pool(name="x", bufs=XBUFS))
        pp = ctx.enter_context(tc.tile_pool(name="ps", bufs=PSBUFS, space="PSUM"))
        op = ctx.enter_context(tc.tile_pool(name="o", bufs=OBUFS))

        # Clock-gate warmup: dummy DoubleRow matmuls on zeros, no DMA deps.
        warm_sb = warmp.tile([P_BLK, 2, 512], f8e4)
        nc.gpsimd.memset(warm_sb[:], 0)
        ps_warm = pp.tile([P_BLK, 512], f32, tag="ps")
        for _ in range(WARMUP_MM):
            nc.tensor.matmul(
                ps_warm[:],
                lhsT=warm_sb[:, :, 0:P_BLK],
                rhs=warm_sb[:],
                start=True,
                stop=True,
                perf_mode=DR,
            )
        # ACT-table preload so the first scalar-engine drain isn't delayed.
        act_sb = actp.tile([1, 1], f32)
        nc.gpsimd.memset(act_sb[:], 0)
        nc.scalar.add(act_sb[:], act_sb[:], 0.0)

        x_tiles: dict[int, object] = {}

        def fetch_chunk(c: int):
            if c not in x_tiles:
                wd = widths[c]
                t = xp.tile([P_BLK, 2, 2, 2, wd], f8e4, tag="x")
                nc.sync.dma_start(t[:], xT.ap()[c][:, :, :, :, 0:wd])
                x_tiles[c] = t

        w_sb = wp.tile([P_BLK, 2, 2, 2, OUT_CH], f8e4)
        # issue order: w stream-0, mini chunk, w stream-1, bias, group 1 —
        # the mini group's (s,j)-outer order needs only w[s=0] + 72KB of x
        # for its first 8 matmuls.
        nc.sync.dma_start(w_sb[:, 0], w8.ap()[:, 0])
        fetch_chunk(0)
        nc.sync.dma_start(w_sb[:, 1], w8.ap()[:, 1])
        b_sb = bp.tile([P_BLK, OBLKS], f32)
        nc.sync.dma_start(b_sb[:], bias2.ap())

        drain_flip = [0]

        def drain(o_sb, oblk, ps_ap):
            bias_ap = b_sb[:, oblk: oblk + 1]
            if drain_flip[0] % 2 == 0:
                nc.vector.tensor_scalar_add(o_sb[:, oblk, :], ps_ap, bias_ap)
            else:
                nc.scalar.add(o_sb[:, oblk, :], ps_ap, bias_ap)
            drain_flip[0] += 1

        for c in range(nchunks):
            fetch_chunk(c)
            for cn in range(c + 1, min(c + 3, nchunks)):
                fetch_chunk(cn)
            wd = widths[c]
            goff = offs[c]
            xt = x_tiles[c]
            o_sb = op.tile([P_BLK, OBLKS, wd], bf16, tag="o")
            if c == 0:
                # Mini chunk: (s,j)-outer over all 4 oblk psum slices so the
                # second w half gets extra time to land.  Slices sit at
                # 512-element offsets (psum-bank aligned).
                psA = pp.tile([P_BLK, 2 * 512], f32, tag="ps")
                psB = pp.tile([P_BLK, 2 * 512], f32, tag="ps")
                mslice = lambda oblk: (psA if oblk < 2 else psB)[
                    :, (oblk % 2) * 512: (oblk % 2) * 512 + wd
                ]
                for s in range(2):
                    for j in range(2):
                        for oblk in range(OBLKS):
                            nc.tensor.matmul(
                                mslice(oblk),
                                lhsT=w_sb[:, s, j, :, oblk * P_BLK:(oblk + 1) * P_BLK],
                                rhs=xt[:, s, j, :, 0:wd],
                                start=(s == 0 and j == 0),
                                stop=(s == 1 and j == 1),
                                perf_mode=DR,
                            )
                for oblk in range(OBLKS):
                    drain(o_sb, oblk, mslice(oblk))
            else:
                for oblk in range(OBLKS):
                    ps = pp.tile([P_BLK, wd], f32, tag="ps")
                    for s in range(2):
                        for j in range(2):
                            lhsT = w_sb[:, s, j, :, oblk * P_BLK:(oblk + 1) * P_BLK]
                            for h in range(0, wd, 512):
                                he = min(h + 512, wd)
                                nc.tensor.matmul(
                                    ps[:, h:he],
                                    lhsT=lhsT,
                                    rhs=xt[:, s, j, :, h:he],
                                    start=(s == 0 and j == 0),
                                    stop=(s == 1 and j == 1),
                                    perf_mode=DR,
                                )
                    drain(o_sb, oblk, ps[:])
            nc.sync.dma_start(out2.ap()[:, :, goff:goff + wd], o_sb[:])
    _legalize_waits(nc)
    return nc


def _get_compiled(plan_key) -> bass.Bass:
    key = (tuple(plan_key[0]), plan_key[2])
    if key not in _compile_cache:
        _compile_cache[key] = _build_bass(plan_key)
    return _compile_cache[key]


def _qe4(a):
    return np.clip(a, -224.0, 224.0).astype(ml_dtypes.float8_e4m3)


def kernel(x, edge_index, node_types, W, b):
    global LAST_RESULTS
    x = np.asarray(x, dtype=np.float32)
    nt = np.asarray(node_types).astype(np.int64)
    W = np.asarray(W, dtype=np.float32)
    b = np.asarray(b, dtype=np.float32)
    N = x.shape[0]

    # Route nodes: stable sort by type, split each type across 2 cores.
    order = np.argsort(nt, kind="stable")
    counts = np.bincount(nt, minlength=NUM_TYPES)
    shards = []
    start = 0
    for t in range(NUM_TYPES):
        c = int(counts[t])
        idx = order[start: start + c]
        start += c
        h = (c + 1) // 2
        shards.append(idx[:h])
        shards.append(idx[h:])

    P_needed = max(1, max(len(g) for g in shards))
    plan = _plan(P_needed)
    widths, offs, P = plan
    nchunks = len(widths)

    nc = _get_compiled(plan)

    # Per-type quantized weights (shared by the 2 cores of each type).
    w_packed = []
    b_packed = []
    for t in range(NUM_TYPES):
        Wq = _qe4(W[t] * SW)                      # [O, K] e4m3
        What = Wq.astype(np.float32) / SW
        V = _qe4((What + (W[t] - What) / ALPHA) * SW)
        # [p, s, j, i, o]: Wq/V [o, kappa] -> kappa = j*256 + i*128 + p
        wp = np.empty((P_BLK, 2, 2, 2, OUT_CH), ml_dtypes.float8_e4m3)
        for s, Ws in enumerate((Wq, V)):
            # Ws.T [K, O] -> [j, i, p, O]
            wt = Ws.T.reshape(2, 2, P_BLK, OUT_CH)
            wp[:, s] = wt.transpose(2, 0, 1, 3)
        w_packed.append(wp)
        b_packed.append(
            np.ascontiguousarray((b[t] * SCALE).reshape(OBLKS, P_BLK).T.astype(np.float32))
        )

    in_maps = []
    for gi, g in enumerate(shards):
        t = gi // 2
        xs = np.zeros((P, IN_CH), np.float32)
        if len(g):
            xs[: len(g)] = x[g]
        m1 = _qe4((1 - ALPHA) * xs * SM)
        xhat = m1.astype(np.float32) / ((1 - ALPHA) * SM)
        u = _qe4((xs - xhat + ALPHA * xhat) * SM)
        xbuf = np.zeros((nchunks, P_BLK, 2, 2, 2, GRP_N), ml_dtypes.float8_e4m3)
        for s, Xs in enumerate((m1, u)):
            XsT = Xs.T.reshape(2, 2, P_BLK, P)  # [j, i, p, node]
            for c in range(nchunks):
                wd = widths[c]
                xbuf[c, :, s, :, :, :wd] = XsT[
                    :, :, :, offs[c]: offs[c] + wd
                ].transpose(2, 0, 1, 3)
        in_maps.append({"xT": xbuf, "w8": w_packed[t], "bias2": b_packed[t]})

    res = run_bass_kernel_spmd(nc, in_maps, list(range(N_CORES)), trace=TRACE)
    LAST_RESULTS = res

    inv = np.float32(1.0 / SCALE)
    out = np.empty((N, OUT_CH), np.float32)
    for gi, g in enumerate(shards):
        if len(g):
            # out2 [128, 4, P] -> [P, 512] node-major (channel = oblk*128+p)
            o = res.results[gi]["out2"].astype(np.float32).transpose(2, 1, 0)
            out[g] = o.reshape(P, OUT_CH)[: len(g)] * inv
    return out
